# revision 48
# baseline (speedup 1.0000x reference)
"""Trainium2 Bass kernel for nn_Attention (dense transformer cross-attention).

Strategy: data-parallel over batch (B=8) -> one batch element per NeuronCore.
Per core, zero on-chip transposes by choosing layouts up front (host
pre-transposes activations/weights, which is free):

  K^T_h [dh=128, M]   = Wk-chunk^T . memory^T   (per head h, bias via DVE)
  Q^T_h [dh=128, Q]   = (scaled Wq)-chunk^T . query^T (bias via DVE)
  V     [M, D]        = memory . Wv^T + bv      (natural layout)
  S^T   [M, Q]        = K^T_h chunk (stationary) . Q^T_h
  expS  = ACT Exp with per-partition mask bias (-1e30 -> exact 0), bf16
  sum_q = DVE tree-reduce the exp chunks to ONE [128, Q] chunk, then a
          single ones-stationary matmul (cross-partition sum)
  1/sum = split bounce chain: part A (row copy + DRAM compaction to
          [128, 8]) kicked at chunk 2 of the next slot; part B (DVE
          reciprocal + DRAM partition-broadcast, bf16) at slot end so no
          DVE work ever queues behind the DMA round trip
  ctx^T_h [dh, Q]     = V-chunks . expS  (PSUM accum over m-chunks),
                        drained unnormalized to SBUF immediately (craw,
                        DVE) so the single PV psum bank recycles without
                        waiting on 1/sum; the rb multiply (latemult)
                        happens a slot later when rb has long landed
  out   [Q, D]        = ctx^T (as lhsT, heads = contraction chunks) . Wf^T
                        + bf via DVE, bf16 out, stores split across the
                        scalar/sync/gpsimd rings (host upcasts)

Software pipeline: slot h runs scores+exp of head h, PV of head h-1,
the 1/sum chain of head h-1 (A at c2, B at end), latemult of head h-2,
and (chunk-paced) the K/Q projections of head h+1.  V projections run
up front — the first (mem, wv) chunk is streamed as two half tiles so
the first matmul starts ~3us earlier; (qt, wq) stream as chunk pairs.
Head 7's reduce tree is pre-computed inside slot 7 so the tail chain
starts immediately.  ACT runs exps only (biases/copies live on DVE);
the sum's [1, Q] row copy is the only other DVE consumer of PSUM.
p0 (= head-0 softmax weights, the second output) is computed on DVE and
streamed out on the sync ring during slot 2.

Masked memory positions produce exactly-zero softmax weights, so m-chunks
that are fully masked in EVERY batch are skipped entirely (K/V projection,
scores, exp, PV, p0); the host zero-fills those output columns.  For the
reference's fixed mask (last quarter masked) this drops 2 of 8 chunks.
A phased (v2) builder serves mask patterns with more than 6 live chunks.

Softmax max-subtraction is skipped: scores are O(1) by construction
(0.02-scale weights), exp is computed in f32 on ACT, so this is exact.

Timing journey (mean-of-8-cores, traced): 197.8us staged baseline ->
~186us: tree-reduced sums (-8us PE), early-release PV drains, split
1/sum chain off the DVE critical path, 3-ring half-tile wm stores,
split first V chunk, pre-computed head-7 tree, front-loaded K/Q filler
pacing.  Run-to-run throttle variance is +/-5%; compare means of
repeated runs.  Measured dead ends (all slower): scalar-ring input or
broadcast DMAs mid-kernel (stalls ACT, the slot pacer), any phase-3
restructure away from qc-outer h-inner streaks, same-slot PV, per-chunk
wk/qt/wq tiles (descriptor issue cost), gpsimd partition_all_reduce
(unsupported ISA here), DVE cross-partition-offset ops (verifier).
"""

import math

import numpy as np
import ml_dtypes

B = 8
Q = 1024
M = 1024
D = 1024
H = 8
DH = 128
KC = 8  # 128-row contraction chunks per 1024
FT = 512
NT = 2

_BF16 = ml_dtypes.bfloat16
_CACHE = {}


def _mtiles(n):
    """Split free dim n into tiles of <= 512."""
    out = []
    o = 0
    while o < n:
        w = min(FT, n - o)
        out.append(slice(o, o + w))
        o += w
    return out


def _build_program(mc):
    """mc = number of live m-chunks (each 128 memory positions)."""
    import concourse.bass as bass
    import concourse.mybir as mybir
    from concourse.tile import TileContext

    import bass_rust

    f32 = mybir.dt.float32
    bf16 = mybir.dt.bfloat16
    Identity = mybir.ActivationFunctionType.Identity
    Exp = mybir.ActivationFunctionType.Exp

    ME = mc * DH  # effective memory length

    def split_sync_waits(nc):
        """The walrus in this container accepts only ONE sync-wait per
        instruction; Tile freely attaches several. Move excess waits onto
        same-engine NOPs spliced immediately before the instruction."""
        for fn in nc.m.functions:
            for bb in fn.blocks:
                out = []
                for inst in bb.instructions:
                    si = inst.sync_info
                    if si is not None and si.on_wait is not None and len(si.on_wait) > 1:
                        waits = list(si.on_wait)
                        si.on_wait = waits[-1:]
                        for j, w in enumerate(waits[:-1]):
                            nop = bass_rust.InstNoOp(
                                name=f"{inst.name}_sw{j}", ins=[], outs=[])
                            nop.engine = inst.engine
                            nop.sync_info = mybir.SyncInfo(on_wait=[w], on_update=[])
                            out.append(nop)
                    out.append(inst)
                bb.instructions = out

    nc = bass.Bass()

    memT = nc.declare_dram_parameter("memT", [D, ME], bf16, isOutput=False)
    qT = nc.declare_dram_parameter("qT", [D, Q], bf16, isOutput=False)
    wkT = nc.declare_dram_parameter("wkT", [D, D], bf16, isOutput=False)
    wvT = nc.declare_dram_parameter("wvT", [D, D], bf16, isOutput=False)
    wqT = nc.declare_dram_parameter("wqT", [D, D], bf16, isOutput=False)
    wfT = nc.declare_dram_parameter("wfT", [D, D], bf16, isOutput=False)
    bk_pp = nc.declare_dram_parameter("bk_pp", [128, H], f32, isOutput=False)
    bq_pp = nc.declare_dram_parameter("bq_pp", [128, H], f32, isOutput=False)
    mb_pp = nc.declare_dram_parameter("mb_pp", [128, mc], f32, isOutput=False)
    bv_bc = nc.declare_dram_parameter("bv_bc", [128, D], bf16, isOutput=False)
    bf_bc = nc.declare_dram_parameter("bf_bc", [128, D], bf16, isOutput=False)

    wm = nc.declare_dram_parameter("wm", [Q, D], bf16, isOutput=True)
    p0t = nc.declare_dram_parameter("p0t", [ME, Q], bf16, isOutput=True)

    def chunked(dram_ap):
        # [1024, N] DRAM -> [p=128, c=8, N] access pattern
        return dram_ap.rearrange("(c p) n -> p c n", p=128)

    m_tiles = _mtiles(ME)

    with TileContext(nc) as tc:
      with tc.tile_pool(name="const", bufs=1) as const, \
           tc.tile_pool(name="persist", bufs=1) as persist:
        bkt = const.tile([128, H], f32)
        bqt = const.tile([128, H], f32)
        mbt = const.tile([128, mc], f32)
        bvt = const.tile([128, D], bf16)
        bft = const.tile([128, D], bf16)
        ones128 = const.tile([128, 128], bf16)
        warm = const.tile([128, 1], f32)
        wf_sb = const.tile([128, KC, D], bf16)

        nc.scalar.dma_start(out=bkt[:], in_=bk_pp[:, :])
        nc.scalar.dma_start(out=bqt[:], in_=bq_pp[:, :])
        nc.scalar.dma_start(out=mbt[:], in_=mb_pp[:, :])
        nc.scalar.dma_start(out=bvt[:], in_=bv_bc[:, :])
        nc.scalar.dma_start(out=bft[:], in_=bf_bc[:, :])
        nc.vector.memset(ones128[:], 1.0)
        # pre-load the ACT exp table set before the first real exp
        nc.scalar.activation(warm[:], bkt[:, 0:1], Exp)

        v_sb = persist.tile([128, mc, D], bf16)
        ctx = [persist.tile([128, Q], bf16, name=f"ctx{h}") for h in range(H)]

        with tc.tile_pool(name="attn", bufs=2) as attn, \
             tc.tile_pool(name="kq", bufs=3) as kq, \
             tc.tile_pool(name="attn3", bufs=2) as attn3, \
             tc.tile_pool(name="dramp", bufs=2, space="DRAM") as dramp, \
             tc.tile_pool(name="ppsum", bufs=1, space="PSUM") as ppsum, \
             tc.tile_pool(name="spsum", bufs=2, space="PSUM") as spsum, \
             tc.tile_pool(name="cpsum", bufs=1, space="PSUM") as cpsum:

          k_t = {}
          q_t = {}
          craw = {}

          def emit_K(h):
              """Generator: yields once per contraction chunk emitted."""
              hs = slice(h * DH, (h + 1) * DH)
              ps = ppsum.tile([128, Q], f32, tag="pp", name=f"kp{h}")
              for c in range(KC):
                  for ts_ in m_tiles:
                      mv = _mem0(ts_) if c == 0 else mem_c[c][:, ts_]
                      nc.tensor.matmul(
                          ps[:, ts_], wk_sb[:, c, hs], mv,
                          start=(c == 0), stop=(c == KC - 1))
                  yield
              k_t[h] = kq.tile([128, ME], bf16, tag="k", name=f"k{h}")
              nc.vector.tensor_scalar_add(k_t[h][:], ps[:, 0:ME],
                                          bkt[:, h:h + 1])
              yield

          def emit_Q(h):
              hs = slice(h * DH, (h + 1) * DH)
              ps = ppsum.tile([128, Q], f32, tag="pp", name=f"qp{h}")
              for c in range(KC):
                  for t in range(NT):
                      ts_ = slice(t * FT, (t + 1) * FT)
                      nc.tensor.matmul(
                          ps[:, ts_], wq_sb[:, c, hs], qt_sb[:, c, ts_],
                          start=(c == 0), stop=(c == KC - 1))
                  yield
              q_t[h] = kq.tile([128, Q], bf16, tag="q", name=f"q{h}")
              nc.vector.tensor_scalar_add(q_t[h][:], ps[:], bqt[:, h:h + 1])
              yield

          def drain_gen(g, n=100000):
              for _ in range(n):
                  if g is None:
                      return None
                  try:
                      next(g)
                  except StopIteration:
                      return None
              return g

          def _mem0(ms):
              if ms.stop <= m0sp:
                  return m0a[:, ms]
              return m0b[:, ms.start - m0sp:ms.stop - m0sp]

          def _wv0(ts_):
              if ts_.stop <= FT:
                  return wv0a[:, ts_]
              return wv0b[:, ts_.start - FT:ts_.stop - FT]

          def emit_V(mcc):
              ms = slice(mcc * DH, (mcc + 1) * DH)
              ps = spsum.tile([128, Q], f32, tag="st", name=f"vp{mcc}")
              for c in range(KC):
                  for t in range(NT):
                      ts_ = slice(t * FT, (t + 1) * FT)
                      if c == 0:
                          nc.tensor.matmul(
                              ps[:, ts_], _mem0(ms), _wv0(ts_),
                              start=True, stop=False)
                      else:
                          nc.tensor.matmul(
                              ps[:, ts_], mem_c[c][:, ms], wv_c[c][:, ts_],
                              start=False, stop=(c == KC - 1))
              nc.vector.tensor_add(v_sb[:, mcc, :], ps[:], bvt[:])

          def emit_red(h, exp_sb):
              """DVE tree-reduce the mc exp chunks down to ONE [128, Q]
              chunk; returns the accumulated chunk AP (red[:, 0, :])."""
              n2 = mc // 2
              nred = n2 + mc % 2
              red = attn.tile([128, nred, Q], bf16, tag="red", bufs=1,
                              name=f"red{h}")
              nc.vector.tensor_add(
                  red[:, 0:n2, :], exp_sb[:, 0:2 * n2:2, :],
                  exp_sb[:, 1:2 * n2:2, :])
              if mc % 2:
                  nc.vector.tensor_copy(red[:, n2, :], exp_sb[:, mc - 1, :])
              while nred > 1:
                  half = nred // 2
                  for i in range(half):
                      nc.vector.tensor_add(
                          red[:, i, :], red[:, i, :], red[:, nred - 1 - i, :])
                  nred = nred - half
              return red[:, 0, :]

          def emit_sum_a(h, acc):
              """Part A: ones-stationary matmul partition sum -> [1, Q]
              row -> DRAM bounce to a compact [128, 8] layout.  Emitted
              early in the slot so the DMA round trip overlaps."""
              sum_ps = spsum.tile([128, Q], f32, tag="st", name=f"sum{h}")
              for t in range(NT):
                  ts_ = slice(t * FT, (t + 1) * FT)
                  nc.tensor.matmul(
                      sum_ps[32 * t:32 * t + 1, 0:FT], ones128[:, 0:1],
                      acc[:, ts_], start=True, stop=True)
              srow = attn.tile([33, FT], bf16, tag="srow", bufs=1,
                               name=f"srow{h}")
              nc.vector.tensor_copy(srow[:], sum_ps[0:33, 0:FT])
              srow_d = dramp.tile([1, Q], bf16, tag="srow_d")
              nc.sync.dma_start(
                  out=srow_d[:, :].rearrange("a (p c) -> (a p) c", p=NT),
                  in_=srow[0:33:32, :])
              comp = attn.tile([128, Q // 128], bf16, tag="comp")
              nc.sync.dma_start(
                  out=comp[:],
                  in_=srow_d[:, :].rearrange("a (p c) -> (a p) c", p=128))
              return comp

          def emit_sum_b(h, comp):
              """Part B: DVE reciprocal -> DRAM bounce partition-
              broadcast back to [128, Q].  Emitted at slot end so no DVE
              work queues behind the comp DMA wait."""
              rcomp = attn.tile([128, Q // 128], f32, tag="rcomp")
              nc.vector.reciprocal(rcomp[:], comp[:])
              rcb = attn.tile([128, Q // 128], bf16, tag="rcb")
              nc.vector.tensor_copy(rcb[:], rcomp[:])
              rrow_d = dramp.tile([1, Q], bf16, tag="rrow_d")
              nc.sync.dma_start(
                  out=rrow_d[:, :].rearrange("a (p c) -> (a p) c", p=128),
                  in_=rcb[:])
              rb = attn.tile([128, Q], bf16,
                             tag=("rb0" if h == 0 else "rb"),
                             bufs=(1 if h == 0 else 2), name=f"rb{h}")
              nc.sync.dma_start(
                  out=rb[:], in_=rrow_d[:, :].partition_broadcast(128))
              return rb

          def emit_pv_pair(ph, pexp, cp, cc):
              phs = slice(ph * DH, (ph + 1) * DH)
              for t in range(NT):
                  ts_ = slice(t * FT, (t + 1) * FT)
                  nc.tensor.matmul(
                      cp[:, ts_], v_sb[:, cc, phs], pexp[:, cc, ts_],
                      start=(cc == 0), stop=(cc == mc - 1))

          def emit_craw(ph, cp):
              """Drain PV psum to SBUF immediately (no 1/sum dependency)
              so the single cpsum buffer frees for the next head."""
              craw[ph] = attn.tile([128, Q], bf16, tag="craw", bufs=2,
                                   name=f"craw{ph}")
              nc.vector.tensor_copy(craw[ph][:], cp[:])

          def emit_latemult(ph):
              nc.vector.tensor_mul(ctx[ph][:], craw[ph][:], rbs[ph][:])
              del craw[ph]

          def emit_p0(pexp, rb, lo, hi):
              # half the chunks per slot: the DVE burst never delays the
              # next slot's K/Q bias adds queued behind it
              for c in range(lo, hi):
                  p0_sb = attn3.tile([128, Q], bf16, tag="p0")
                  nc.vector.tensor_mul(p0_sb[:], pexp[:, c, :], rb[:])
                  nc.gpsimd.dma_start(
                      out=p0t[c * DH:(c + 1) * DH, :], in_=p0_sb[:])

          with tc.tile_pool(name="io1", bufs=1) as io1, \
               tc.tile_pool(name="io2", bufs=1) as io2:
            mem_c = [io1.tile([128, ME], bf16, name=f"mem{c}")
                     for c in range(KC)]
            m0sp = min(FT, ME)
            m0a = io1.tile([128, m0sp], bf16, name="m0a")
            m0b = io1.tile([128, max(ME - m0sp, 1)], bf16, name="m0b")
            wv0a = io2.tile([128, FT], bf16, name="wv0a")
            wv0b = io2.tile([128, D - FT], bf16, name="wv0b")
            qt_sb = io1.tile([128, KC, Q], bf16)
            wk_sb = io1.tile([128, KC, D], bf16)
            wq_sb = io1.tile([128, KC, D], bf16)
            wv_c = [io2.tile([128, D], bf16, name=f"wv{c}") for c in range(KC)]
            # V inputs first, chunk-interleaved: V projections start as soon
            # as the first (mem, wv) chunk pair lands.
            nc.sync.dma_start(out=m0a[:], in_=chunked(memT[:, :])[:, 0, 0:m0sp])
            nc.sync.dma_start(out=wv0a[:], in_=chunked(wvT[:, :])[:, 0, 0:FT])
            if ME > m0sp:
                nc.sync.dma_start(out=m0b[:], in_=chunked(memT[:, :])[:, 0, m0sp:ME])
            nc.sync.dma_start(out=wv0b[:], in_=chunked(wvT[:, :])[:, 0, FT:D])
            for c in range(1, KC):
                nc.sync.dma_start(out=mem_c[c][:], in_=chunked(memT[:, :])[:, c, :])
                nc.sync.dma_start(out=wv_c[c][:], in_=chunked(wvT[:, :])[:, c, :])
            nc.sync.dma_start(out=wk_sb[:], in_=chunked(wkT[:, :]))
            # (qt, wq) as chunk pairs so the Q projection accumulates
            # chunk-paced as the stream lands, like V does.
            for c in range(KC):
                nc.sync.dma_start(out=qt_sb[:, c, :], in_=chunked(qT[:, :])[:, c, :])
                nc.sync.dma_start(out=wq_sb[:, c, :], in_=chunked(wqT[:, :])[:, c, :])
            # wf is only needed by phase 3; last on the sync ring so it
            # never competes with the critical input stream.
            nc.sync.dma_start(out=wf_sb[:], in_=chunked(wfT[:, :]))

            for mcc in range(mc):
                emit_V(mcc)
            drain_gen(emit_K(0))
            drain_gen(emit_Q(0))

            exps = {}
            rbs = {}
            pending = None  # head awaiting sum+bounce
            acc_prev = None
            comp_prev = None
            for h in range(H):
                exp_sb = attn.tile([128, mc, Q], bf16,
                                   tag=("exp0" if h == 0 else "expS"),
                                   bufs=(1 if h == 0 else 2),
                                   name=f"exp{h}")
                exps[h] = exp_sb
                cp = (cpsum.tile([128, Q], f32, tag="cp", name=f"cp{h - 1}")
                      if h >= 1 else None)
                filler = None
                if h + 1 < H:
                    def _fill(hh):
                        yield from emit_K(hh)
                        yield from emit_Q(hh)
                    filler = _fill(h + 1)
                for c in range(mc):
                    cs = slice(c * DH, (c + 1) * DH)
                    st = spsum.tile([128, Q], f32, tag="st")
                    for t in range(NT):
                        ts_ = slice(t * FT, (t + 1) * FT)
                        nc.tensor.matmul(
                            st[:, ts_], k_t[h][:, cs], q_t[h][:, ts_],
                            start=True, stop=True)
                    if pending is not None and c == 0:
                        acc_prev = emit_red(pending, exps[pending])
                    nc.scalar.activation(
                        exp_sb[:, c, :], st[:], Exp, bias=mbt[:, c:c + 1])
                    if h >= 1:
                        emit_pv_pair(h - 1, exps[h - 1], cp, c)
                    if pending is not None and c == 2:
                        # kick the sum + compaction DMA round trip early;
                        # the reciprocal half runs at slot end
                        comp_prev = emit_sum_a(pending, acc_prev)
                    if h == H - 1 and c == mc - 1 and mc >= 5:
                        # pre-reduce head 7's first four exp chunks so the
                        # tail tree only needs the last chunks
                        red7 = attn.tile([128, 2, Q], bf16, tag="red",
                                         bufs=1, name="red7pre")
                        nc.vector.tensor_add(
                            red7[:, 0:2, :], exp_sb[:, 0:4:2, :],
                            exp_sb[:, 1:4:2, :])
                    filler = drain_gen(filler, 5 if c < mc // 2 else 3)
                filler = drain_gen(filler)
                if h == H - 1:
                    # last slot: finish head 7's tree first so the tail
                    # chain starts before this slot's trailing DVE ops
                    if mc >= 5:
                        nc.vector.tensor_add(
                            red7[:, 0, :], red7[:, 0, :], red7[:, 1, :])
                        for cc in range(4, mc):
                            nc.vector.tensor_add(
                                red7[:, 0, :], red7[:, 0, :],
                                exp_sb[:, cc, :])
                        acc_last = red7[:, 0, :]
                    else:
                        acc_last = emit_red(h, exps[h])
                if h >= 1:
                    emit_craw(h - 1, cp)
                if h >= 2:
                    emit_latemult(h - 2)
                    if h == 2:
                        emit_p0(exps[0], rbs[0], 0, mc // 2)
                    elif h == 3:
                        emit_p0(exps[0], rbs[0], mc // 2, mc)
                        del exps[0]
                        del exps[1]
                    if h >= 4:
                        del exps[h - 2]
                if pending is not None:
                    rbs[pending] = emit_sum_b(pending, comp_prev)
                    pending = None
                pending = h

            # tail: kick head 7's chain interleaved with PV(7) so the PE
            # covers the reciprocal round trip
            cp = cpsum.tile([128, Q], f32, tag="cp", name=f"cp{H - 1}")
            for c in range(2):
                emit_pv_pair(H - 1, exps[H - 1], cp, c)
            comp_last = emit_sum_a(pending, acc_last)
            for c in range(2, mc):
                emit_pv_pair(H - 1, exps[H - 1], cp, c)
            # latemult(6) first: its inputs landed at slot-7 end, while
            # craw7 must wait for PV(7)'s psum -- keeping ctx6 off the
            # phase-3 critical path
            emit_latemult(H - 2)
            emit_craw(H - 1, cp)
            rbs[pending] = emit_sum_b(pending, comp_last)
            emit_latemult(H - 1)

        # ---------------- Phase 3: final projection ----------------
        # qc0 accumulates h<=6 then PARKS; its h==7 closer runs at the
        # very end.  qc1..qc7 stream their full h0..7 groups meanwhile,
        # so the PE never waits in-order on head 7's late-mult.
        with tc.tile_pool(name="fin", bufs=3) as fin, \
             tc.tile_pool(name="fpsum", bufs=4, space="PSUM") as fpsum:
            out_engs = [nc.scalar, nc.sync, nc.gpsimd]
            ei = 0

            def fp_head(fp, qs, h, stop):
                for t in range(NT):
                    ts_ = slice(t * FT, (t + 1) * FT)
                    nc.tensor.matmul(
                        fp[:, ts_], ctx[h][:, qs], wf_sb[:, h, ts_],
                        start=(h == 0), stop=stop)

            def fp_store(fp, qs):
                nonlocal ei
                of = fin.tile([128, D], bf16, tag="of")
                for t in range(NT):
                    ts_ = slice(t * FT, (t + 1) * FT)
                    nc.vector.tensor_add(of[:, ts_], fp[:, ts_], bft[:, ts_])
                    for half in range(2):
                        hs_ = slice(t * FT + half * (FT // 2),
                                    t * FT + (half + 1) * (FT // 2))
                        eng = out_engs[ei % 3]
                        ei += 1
                        eng.dma_start(out=wm[qs, hs_], in_=of[:, hs_])

            qs0 = slice(0, DH)
            fp0 = fpsum.tile([128, Q], f32, tag="fp", name="fp_qc0")
            for h in range(H - 1):
                fp_head(fp0, qs0, h, stop=False)
            for qc in range(1, KC):
                qs = slice(qc * DH, (qc + 1) * DH)
                fp = fpsum.tile([128, Q], f32, tag="fp", name=f"fp_qc{qc}")
                for h in range(H):
                    fp_head(fp, qs, h, stop=(h == H - 1))
                fp_store(fp, qs)
            fp_head(fp0, qs0, H - 1, stop=True)
            fp_store(fp0, qs0)

    split_sync_waits(nc)
    return nc


def _build_program_phased(mc):
    """mc = number of live m-chunks (each 128 memory positions)."""
    import concourse.bass as bass
    import concourse.mybir as mybir
    from concourse.tile import TileContext

    import bass_rust

    f32 = mybir.dt.float32
    bf16 = mybir.dt.bfloat16
    Identity = mybir.ActivationFunctionType.Identity
    Exp = mybir.ActivationFunctionType.Exp

    ME = mc * DH  # effective memory length

    def split_sync_waits(nc):
        """The walrus in this container accepts only ONE sync-wait per
        instruction; Tile freely attaches several. Move excess waits onto
        same-engine NOPs spliced immediately before the instruction."""
        for fn in nc.m.functions:
            for bb in fn.blocks:
                out = []
                for inst in bb.instructions:
                    si = inst.sync_info
                    if si is not None and si.on_wait is not None and len(si.on_wait) > 1:
                        waits = list(si.on_wait)
                        si.on_wait = waits[-1:]
                        for j, w in enumerate(waits[:-1]):
                            nop = bass_rust.InstNoOp(
                                name=f"{inst.name}_sw{j}", ins=[], outs=[])
                            nop.engine = inst.engine
                            nop.sync_info = mybir.SyncInfo(on_wait=[w], on_update=[])
                            out.append(nop)
                    out.append(inst)
                bb.instructions = out

    nc = bass.Bass()

    memT = nc.declare_dram_parameter("memT", [D, ME], bf16, isOutput=False)
    qT = nc.declare_dram_parameter("qT", [D, Q], bf16, isOutput=False)
    wkT = nc.declare_dram_parameter("wkT", [D, D], bf16, isOutput=False)
    wvT = nc.declare_dram_parameter("wvT", [D, D], bf16, isOutput=False)
    wqT = nc.declare_dram_parameter("wqT", [D, D], bf16, isOutput=False)
    wfT = nc.declare_dram_parameter("wfT", [D, D], bf16, isOutput=False)
    bk_pp = nc.declare_dram_parameter("bk_pp", [128, H], f32, isOutput=False)
    bq_pp = nc.declare_dram_parameter("bq_pp", [128, H], f32, isOutput=False)
    mb_pp = nc.declare_dram_parameter("mb_pp", [128, mc], f32, isOutput=False)
    bv_bc = nc.declare_dram_parameter("bv_bc", [128, D], bf16, isOutput=False)
    bf_bc = nc.declare_dram_parameter("bf_bc", [128, D], bf16, isOutput=False)

    wm = nc.declare_dram_parameter("wm", [Q, D], bf16, isOutput=True)
    p0t = nc.declare_dram_parameter("p0t", [ME, Q], bf16, isOutput=True)

    def chunked(dram_ap, n=None):
        # [1024, N] DRAM -> [p=128, c=8, N] access pattern
        return dram_ap.rearrange("(c p) n -> p c n", p=128)

    m_tiles = _mtiles(ME)

    with TileContext(nc) as tc:
        with tc.tile_pool(name="const", bufs=1) as const, \
             tc.tile_pool(name="persist", bufs=1) as persist:
            wf_sb = const.tile([128, KC, D], bf16)
            bkt = const.tile([128, H], f32)
            bqt = const.tile([128, H], f32)
            mbt = const.tile([128, mc], f32)
            bvt = const.tile([128, D], bf16)
            bft = const.tile([128, D], bf16)
            ones128 = const.tile([128, 128], bf16)
            warm = const.tile([128, 1], f32)

            nc.scalar.dma_start(out=bkt[:], in_=bk_pp[:, :])
            nc.scalar.dma_start(out=bqt[:], in_=bq_pp[:, :])
            nc.scalar.dma_start(out=mbt[:], in_=mb_pp[:, :])
            nc.scalar.dma_start(out=bvt[:], in_=bv_bc[:, :])
            nc.scalar.dma_start(out=bft[:], in_=bf_bc[:, :])
            nc.scalar.dma_start(out=wf_sb[:], in_=chunked(wfT[:, :]))
            nc.vector.memset(ones128[:], 1.0)
            # pre-load the ACT exp table set during phase 1
            nc.scalar.activation(warm[:], bkt[:, 0:1], Exp)

            k_sb = persist.tile([128, H, ME], bf16)
            q_sb = persist.tile([128, H, Q], bf16)
            v_sb = persist.tile([128, mc, D], bf16)
            ctx_sb = persist.tile([128, H, Q], bf16)

            # ---------------- Phase 1: projections ----------------
            with tc.tile_pool(name="proj", bufs=1) as proj, \
                 tc.tile_pool(name="ppsum", bufs=3, space="PSUM") as ppsum:
                mem_sb = proj.tile([128, KC, ME], bf16)
                qt_sb = proj.tile([128, KC, Q], bf16)
                wk_sb = proj.tile([128, KC, D], bf16)
                wv_sb = proj.tile([128, KC, D], bf16)
                wq_sb = proj.tile([128, KC, D], bf16)
                # K-projection inputs stream per chunk pair so PE starts
                # accumulating as data lands; Q inputs next, Wv last.
                for c in range(KC):
                    nc.sync.dma_start(out=wk_sb[:, c, :], in_=chunked(wkT[:, :])[:, c, :])
                    nc.sync.dma_start(out=mem_sb[:, c, :], in_=chunked(memT[:, :])[:, c, :])
                nc.sync.dma_start(out=qt_sb[:], in_=chunked(qT[:, :]))
                nc.sync.dma_start(out=wq_sb[:], in_=chunked(wqT[:, :]))
                nc.sync.dma_start(out=wv_sb[:], in_=chunked(wvT[:, :]))

                for h in range(H):
                    hs = slice(h * DH, (h + 1) * DH)
                    ps = ppsum.tile([128, Q], f32, tag="pp")
                    for c in range(KC):
                        for ts_ in m_tiles:
                            nc.tensor.matmul(
                                ps[:, ts_], wk_sb[:, c, hs], mem_sb[:, c, ts_],
                                start=(c == 0), stop=(c == KC - 1))
                    nc.scalar.activation(
                        k_sb[:, h, :], ps[:, 0:ME], Identity, bias=bkt[:, h:h + 1])
                for h in range(H):
                    hs = slice(h * DH, (h + 1) * DH)
                    ps2 = ppsum.tile([128, Q], f32, tag="pp")
                    for c in range(KC):
                        for t in range(NT):
                            ts_ = slice(t * FT, (t + 1) * FT)
                            nc.tensor.matmul(
                                ps2[:, ts_], wq_sb[:, c, hs], qt_sb[:, c, ts_],
                                start=(c == 0), stop=(c == KC - 1))
                    nc.scalar.activation(
                        q_sb[:, h, :], ps2[:], Identity, bias=bqt[:, h:h + 1])

                for mcc in range(mc):
                    ms = slice(mcc * DH, (mcc + 1) * DH)
                    ps = ppsum.tile([128, Q], f32, tag="pp")
                    for c in range(KC):
                        for t in range(NT):
                            ts_ = slice(t * FT, (t + 1) * FT)
                            nc.tensor.matmul(
                                ps[:, ts_], mem_sb[:, c, ms], wv_sb[:, c, ts_],
                                start=(c == 0), stop=(c == KC - 1))
                    nc.vector.tensor_add(v_sb[:, mcc, :], ps[:], bvt[:])

            # ---------------- Phase 2: attention (per head) ----------------
            with tc.tile_pool(name="attn", bufs=2) as attn, \
                 tc.tile_pool(name="attn3", bufs=2) as attn3, \
                 tc.tile_pool(name="dramp", bufs=2, space="DRAM") as dramp, \
                 tc.tile_pool(name="spsum", bufs=2, space="PSUM") as spsum, \
                 tc.tile_pool(name="cpsum", bufs=2, space="PSUM") as cpsum:

                def emit_sum_and_bounce(h, acc):
                    """Cross-partition sum of acc -> 1/sum broadcast [128,Q].
                    Returns the rb tile (f32, [128, Q])."""
                    sum_ps = spsum.tile([128, Q], f32, tag="st",
                                        name=f"sum_h{h}")
                    for t in range(NT):
                        ts_ = slice(t * FT, (t + 1) * FT)
                        nc.tensor.matmul(
                            sum_ps[:, ts_], ones128[:], acc[:, ts_],
                            start=True, stop=True)
                    srow = attn.tile([1, Q], f32, tag="srow", name=f"srow_h{h}")
                    nc.vector.tensor_copy(srow[:], sum_ps[0:1, :])
                    srow_d = dramp.tile([1, Q], f32, tag="srow_d")
                    nc.sync.dma_start(out=srow_d[:, :], in_=srow[:])
                    comp = attn.tile([128, Q // 128], f32, tag="comp")
                    nc.sync.dma_start(
                        out=comp[:],
                        in_=srow_d[:, :].rearrange("a (p c) -> (a p) c", p=128))
                    rcomp = attn.tile([128, Q // 128], f32, tag="rcomp")
                    nc.vector.reciprocal(rcomp[:], comp[:])
                    rrow_d = dramp.tile([1, Q], f32, tag="rrow_d")
                    nc.sync.dma_start(
                        out=rrow_d[:, :].rearrange("a (p c) -> (a p) c", p=128),
                        in_=rcomp[:])
                    rb = attn.tile([128, Q], f32, tag="rb", name=f"rb_h{h}")
                    nc.sync.dma_start(
                        out=rb[:], in_=rrow_d[:, :].partition_broadcast(128))
                    return rb

                def emit_pv_pair(ph, pexp, cp, cc):
                    phs = slice(ph * DH, (ph + 1) * DH)
                    for t in range(NT):
                        ts_ = slice(t * FT, (t + 1) * FT)
                        nc.tensor.matmul(
                            cp[:, ts_], v_sb[:, cc, phs], pexp[:, cc, ts_],
                            start=(cc == 0), stop=(cc == mc - 1))

                def emit_drains(ph, cp, rb):
                    for t in range(NT):
                        ts_ = slice(t * FT, (t + 1) * FT)
                        nc.vector.tensor_mul(
                            ctx_sb[:, ph, ts_], cp[:, ts_], rb[:, ts_])

                def emit_p0(pexp, rb):
                    for c in range(mc):
                        p0_sb = attn3.tile([128, Q], bf16, tag="p0")
                        nc.gpsimd.tensor_mul(p0_sb[:], pexp[:, c, :], rb[:])
                        nc.sync.dma_start(
                            out=p0t[c * DH:(c + 1) * DH, :], in_=p0_sb[:])

                prev = None          # (h, exp_sb, cp, rb) of previous head
                pending = None       # (h, acc, exp_sb) awaiting sum+bounce
                for h in range(H):
                    hs = slice(h * DH, (h + 1) * DH)
                    exp_sb = attn.tile([128, mc, Q], bf16, tag="expS", bufs=3,
                                       name=f"exp_h{h}")
                    pairs = []
                    cp = None
                    if prev is not None:
                        cp = cpsum.tile([128, Q], f32, tag="cp",
                                        name=f"cp_h{prev[0]}")
                        prev = (prev[0], prev[1], cp)
                    for c in range(mc):
                        cs = slice(c * DH, (c + 1) * DH)
                        st = spsum.tile([128, Q], f32, tag="st")
                        for t in range(NT):
                            ts_ = slice(t * FT, (t + 1) * FT)
                            nc.tensor.matmul(
                                st[:, ts_], k_sb[:, h, cs], q_sb[:, h, ts_],
                                start=True, stop=True)
                        if pending is not None and c == 2:
                            ph, acc = pending
                            rb_prev = emit_sum_and_bounce(ph, acc)
                            pending = None
                        nc.scalar.activation(
                            exp_sb[:, c, :], st[:], Exp, bias=mbt[:, c:c + 1])
                        if prev is not None:
                            emit_pv_pair(prev[0], prev[1], cp, c)
                        if c % 2 == 1:
                            pr = attn.tile([128, Q], bf16, tag=f"pr{c // 2}",
                                           name=f"pr{c // 2}_h{h}")
                            nc.vector.tensor_add(
                                pr[:], exp_sb[:, c - 1, :], exp_sb[:, c, :])
                            pairs.append(pr)
                    if mc % 2 == 1:
                        pairs.append(exp_sb[:, mc - 1, :])
                    # tree over remaining partial pairs -> acc
                    while len(pairs) > 1:
                        nxt = []
                        for i in range(0, len(pairs) - 1, 2):
                            t_ = attn.tile([128, Q], bf16, tag=f"tr{len(pairs)}_{i}",
                                           name=f"tr{len(pairs)}_{i}_h{h}")
                            nc.vector.tensor_add(t_[:], pairs[i][:], pairs[i + 1][:])
                            nxt.append(t_)
                        if len(pairs) % 2 == 1:
                            nxt.append(pairs[-1])
                        pairs = nxt
                    acc = pairs[0]
                    if prev is not None:
                        # prev head's PV is complete; drain with its rb
                        emit_drains(prev[0], cp, rb_prev)
                        if prev[0] == 0:
                            emit_p0(prev[1], rb_prev)
                    pending = (h, acc)
                    prev = (h, exp_sb, None)

                # tail: last head's sum + PV + drains
                ph, acc = pending
                rb_last = emit_sum_and_bounce(ph, acc)
                cp = cpsum.tile([128, Q], f32, tag="cp", name=f"cp_h{ph}")
                for c in range(mc):
                    emit_pv_pair(ph, prev[1], cp, c)
                emit_drains(ph, cp, rb_last)

            # ---------------- Phase 3: final projection ----------------
            with tc.tile_pool(name="fin", bufs=3) as fin, \
                 tc.tile_pool(name="fpsum", bufs=4, space="PSUM") as fpsum:
                for qc in range(KC):
                    qs = slice(qc * DH, (qc + 1) * DH)
                    fp = fpsum.tile([128, Q], f32, tag="fp")
                    for h in range(H):
                        for t in range(NT):
                            ts_ = slice(t * FT, (t + 1) * FT)
                            nc.tensor.matmul(
                                fp[:, ts_], ctx_sb[:, h, qs], wf_sb[:, h, ts_],
                                start=(h == 0), stop=(h == H - 1))
                    of = fin.tile([128, D], bf16, tag="of")
                    nc.vector.tensor_add(of[:], fp[:], bft[:])
                    eng = nc.scalar if qc % 2 == 0 else nc.sync
                    eng.dma_start(out=wm[qs, :], in_=of[:])

    split_sync_waits(nc)
    return nc



def _get_program(mc):
    key = f"mc{mc}"
    if key not in _CACHE:
        # the software-pipelined builder's SBUF footprint and schedule fit
        # the masked-suffix fast path (4-6 live chunks); anything else uses
        # the phased builder
        build = _build_program if 4 <= mc <= 6 else _build_program_phased
        _CACHE[key] = build(mc)
    return _CACHE[key]


def _host_prep(query, memory, mask, Wk, bk, Wv, bv, Wq, bq, Wf, bf, live):
    scale = 1.0 / math.sqrt(DH)
    f32 = np.float32
    mc = len(live)
    # memory positions belonging to live chunks, in chunk order
    live_pos = np.concatenate([np.arange(c * DH, (c + 1) * DH) for c in live])

    def t_bf16(a):
        return np.ascontiguousarray(np.asarray(a, dtype=f32).T).astype(_BF16)

    shared = {
        "wkT": t_bf16(Wk),
        "wvT": t_bf16(Wv),
        "wqT": np.ascontiguousarray(
            np.asarray(Wq, dtype=f32).T * f32(scale)).astype(_BF16),
        "wfT": t_bf16(Wf),
        "bk_pp": np.ascontiguousarray(
            np.asarray(bk, dtype=f32).reshape(H, DH).T),
        "bq_pp": np.ascontiguousarray(
            (np.asarray(bq, dtype=f32) * f32(scale)).reshape(H, DH).T),
        "bv_bc": np.ascontiguousarray(
            np.broadcast_to(np.asarray(bv, dtype=f32), (128, D))).astype(_BF16),
        "bf_bc": np.ascontiguousarray(
            np.broadcast_to(np.asarray(bf, dtype=f32), (128, D))).astype(_BF16),
    }
    mask = np.asarray(mask)
    in_maps = []
    for b in range(B):
        mb = np.where(mask[b, live_pos], f32(-1e30), f32(0.0)).astype(f32)
        in_maps.append({
            **shared,
            "memT": np.ascontiguousarray(
                np.asarray(memory[b], dtype=f32).T[:, live_pos]).astype(_BF16),
            "qT": t_bf16(query[b]),
            "mb_pp": np.ascontiguousarray(mb.reshape(mc, DH).T),
        })
    return in_maps


def kernel(query, memory, mask, Wk, bk, Wv, bv, Wq, bq, Wf, bf):
    from concourse.bass_utils import run_bass_kernel_spmd

    mask_np = np.asarray(mask)
    # chunks of 128 memory positions that are fully masked in EVERY batch
    # contribute exactly zero to softmax numerator/denominator -> skip them
    chunk_dead = mask_np.reshape(B, KC, DH).all(axis=2).all(axis=0)
    live = [c for c in range(KC) if not chunk_dead[c]]
    mc = len(live)

    nc = _get_program(mc)
    in_maps = _host_prep(query, memory, mask, Wk, bk, Wv, bv, Wq, bq, Wf, bf,
                         live)
    res = run_bass_kernel_spmd(nc, in_maps, core_ids=list(range(B)))
    wm = np.stack([res.results[b]["wm"].astype(np.float32) for b in range(B)])
    w0 = np.zeros((B, Q, M), dtype=np.float32)
    for b in range(B):
        p0 = res.results[b]["p0t"].astype(np.float32)  # [ME, Q]
        for i, c in enumerate(live):
            w0[b, :, c * DH:(c + 1) * DH] = p0[i * DH:(i + 1) * DH, :].T
    return wm, w0

